# revision 9
# baseline (speedup 1.0000x reference)
"""GroupedQueryAttention (B=1, S=4096, D=1024, G=16 heads, DH=64) on 8 TRN2 NeuronCores.

Sharding: tensor-parallel over heads. Core c computes heads {2c, 2c+1}:
  - Q/K/V projections with column-sliced weights (128 out-dims per core),
    producing Q^T/K^T in [dout, seq] layout (host pre-transposes inputs).
  - V^T is built per head padded to 80 rows with an all-ones row 64; SBUF->
    SBUF xbar DMA transposes turn it into V-natural [kpos, 80] chunks whose
    col 64 is the ones column that makes the PV matmul accumulate the
    softmax denominator for free.
  - Flash-style attention without max-subtraction (scores are tiny:
    |s/8| < ~3), exp on ScalarE with fused 1/8 scale + per-key mask bias.
  - Output projection with row-sliced Wo produces a partial (4096, 1024)
    bf16 output per core; host sums the 8 partials and adds bo.

Attention engine schedule: the exp stream on ScalarE is the critical path
(2 x [128,1024] ACTIVATEs per 128-wide k-chunk, ~2.2us), so the PE work is
arranged to stay under that pace and never stall the ACT queue:
  - Scores for head0/head1 run as row-tiled (64x128) matmuls on PE tiles
    (0,0) and (64,0) (auto-derived from base partitions) and execute
    CONCURRENTLY on the two 64-row halves of the array.
  - PV runs split-K in the same (64,128) mode (no PE mode-switch drains in
    the attention loop): each accumulator takes a low-k-rows matmul from
    tile (0,0) and a high-k-rows matmul from tile (64,0). Issue order
    T0:[A00 A01 A11 A10] / T8:[A11 A10 A00 A01] keeps any two same-bank
    writes >= 2 matmul slots apart (row tiles must not touch one PSUM bank
    simultaneously).
  - PSUM: 2 score slots x 2 banks + 4 accumulators x 1 bank = 8 banks.
    The output projection / next-quarter Q projection borrow score-pool
    slots in small batches sprinkled between k-chunks of the following
    quarter, so ACT never waits at quarter boundaries.

All matmul operands are bf16 (fp32 PSUM accumulation). K/V inputs stream
through quarter-chunk tiles; Q streams in two quarters ahead.
"""

import os
import sys

for _p in ("/opt/trn_rl_repo", "/root/.axon_site/_ro/trn_rl_repo"):
    if os.path.isdir(_p) and _p not in sys.path:
        sys.path.insert(0, _p)

from contextlib import ExitStack

import ml_dtypes
import numpy as np

import concourse.bass as bass
import concourse.mybir as mybir
import concourse.tile as tile
from concourse import bacc
from concourse.bass_utils import run_bass_kernel_spmd

S = 4096          # sequence length
D = 1024          # model dim
G = 16            # heads
DH = 64           # head dim
P = 128           # partitions
QT = 512          # q-tile (moving free dim)
KC = 128          # k-chunk
NCORES = 8
HPC = G // NCORES             # heads per core = 2
N_ST = S // QT                # 8 s-tiles of 512
N_KCH = D // P                # 8 contraction chunks for projections
N_KC = S // KC                # 32 k-chunks for attention
DSL = P                       # per-core dout slice (2 heads * 64)
QPH = 2                       # q-tiles per attention group (quarter)
N_GRP = N_ST // QPH           # 4 quarters
QC = S // N_GRP               # 1024 columns per quarter chunk

F32 = mybir.dt.float32
BF16 = mybir.dt.bfloat16
BF = ml_dtypes.bfloat16

_CACHE = {}


def _build_nc():
    key = "nc"
    if key in _CACHE:
        return _CACHE[key]

    nc = bacc.Bacc(
        "TRN2", target_bir_lowering=False, debug=False, num_devices=NCORES
    )

    xqT = nc.dram_tensor("xqT", [D, S], BF16, kind="ExternalInput").ap()
    xkT = nc.dram_tensor("xkT", [D, S], BF16, kind="ExternalInput").ap()
    xvT = nc.dram_tensor("xvT", [D, S], BF16, kind="ExternalInput").ap()
    wqT = nc.dram_tensor("wqT", [N_KCH, P, DSL], BF16, kind="ExternalInput").ap()
    wkT = nc.dram_tensor("wkT", [N_KCH, P, DSL], BF16, kind="ExternalInput").ap()
    wvT = nc.dram_tensor("wvT", [N_KCH, P, DSL], BF16, kind="ExternalInput").ap()
    woT = nc.dram_tensor("woT", [DSL, D], BF16, kind="ExternalInput").ap()
    bq = nc.dram_tensor("bq", [DSL, 1], F32, kind="ExternalInput").ap()
    bk = nc.dram_tensor("bk", [DSL, 1], F32, kind="ExternalInput").ap()
    bv = nc.dram_tensor("bv", [DSL, 1], F32, kind="ExternalInput").ap()
    mbias = nc.dram_tensor("mbias", [P, N_KC], F32, kind="ExternalInput").ap()
    out_d = nc.dram_tensor("out", [S, D], BF16, kind="ExternalOutput").ap()

    with tile.TileContext(nc) as tc, ExitStack() as ctx:
        consts = ctx.enter_context(tc.tile_pool(name="consts", bufs=1))
        big = ctx.enter_context(tc.tile_pool(name="big", bufs=1))
        et_pool = ctx.enter_context(tc.tile_pool(name="et", bufs=8))
        small = ctx.enter_context(tc.tile_pool(name="small", bufs=2))
        oevict = ctx.enter_context(tc.tile_pool(name="oevict", bufs=4))
        # streamed K/V quarter-chunk tiles: 20 slots per tensor tag = 2.5
        # quarters of lookahead so chunk DMAs stay ahead of the proj matmuls
        xc = ctx.enter_context(tc.tile_pool(name="xc", bufs=20))
        # streamed Q quarter-chunk tiles, two quarters in flight
        xq = ctx.enter_context(tc.tile_pool(name="xq", bufs=16))
        # PSUM: 'sc' 2 slots x 2 banks (scores; also borrowed by out/q proj)
        #     + 'acc' 4 slots x 1 bank (PV accumulators; phase-1 proj tiles)
        ps_sc = ctx.enter_context(tc.tile_pool(name="ps_sc", bufs=2, space="PSUM"))
        ps_acc = ctx.enter_context(tc.tile_pool(name="ps_acc", bufs=4, space="PSUM"))

        # ---- constants (scalar-queue DMAs, parallel to x loads on sync) ----
        w_s = {}
        for name, wd in (("k", wkT), ("v", wvT), ("q", wqT)):
            w = consts.tile([P, N_KCH * DSL], BF16, tag=f"w{name}")
            for kc in range(N_KCH):
                nc.scalar.dma_start(w[:, kc * DSL:(kc + 1) * DSL], wd[kc])
            w_s[name] = w
        wo_s = consts.tile([DSL, D], BF16, tag="wo")
        nc.scalar.dma_start(wo_s[:], woT)
        b_s = {}
        for name, bd in (("q", bq), ("k", bk), ("v", bv)):
            b = consts.tile([DSL, 1], F32, tag=f"b{name}")
            nc.scalar.dma_start(b[:], bd)
            b_s[name] = b
        mb_s = consts.tile([P, N_KC], F32, tag="mb")
        nc.scalar.dma_start(mb_s[:], mbias)

        def load_quarter(pool, tag, xd, qtr, nsplit=2):
            """Load one quarter's 8 contraction chunks, rotating across
            DMA queues so the streams run in parallel. The scalar queue is
            only borrowed during phase 1 (before the exp stream starts)."""
            engines = [nc.sync, nc.gpsimd, nc.scalar][:nsplit]
            qsl = slice(qtr * QC, (qtr + 1) * QC)
            tiles = []
            for kc in range(N_KCH):
                t = pool.tile([P, QC], BF16, tag=tag, name=f"{tag}{qtr}{kc}")
                engines[kc % nsplit].dma_start(t[:], xd[kc * P:(kc + 1) * P, qsl])
                tiles.append(t)
            return tiles

        # ---- resident activations ----
        QTs = big.tile([P, S], BF16, tag="QTs")      # Q^T  [dout, s]
        KTs = big.tile([P, S], BF16, tag="KTs")      # K^T  [dout, s]
        # V^T per head, padded to 80 rows: rows 0-63 V data, row 64 ones
        # (becomes the PV ones-column after transpose -> softmax denom),
        # rows 65-79 zero pad (xbar transpose needs partition%16==0).
        VTh = [
            big.tile([DH + 16, S], BF16, tag=f"VTh{h}", name=f"VTh{h}")
            for h in range(HPC)
        ]
        # V natural per head: [k-part, chunk, 80]; col DH is the ones column.
        Vnat = [
            big.tile([P, N_KC, DH + 16], BF16, tag=f"Vn{h}", name=f"Vnat{h}")
            for h in range(HPC)
        ]
        attnT = big.tile([P, S], BF16, tag="attnT")  # normalized attn^T [din, s]

        for h in range(HPC):
            nc.vector.memset(VTh[h][DH:DH + 16, :], 0.0)
            nc.vector.memset(VTh[h][DH:DH + 1, :], 1.0)

        def project(name, dst, st, xtiles, stl, pool, tag):
            """One 512-wide s-tile of the `name` projection into dst (bf16)."""
            sl = slice(st * QT, (st + 1) * QT)
            xsl = slice(stl * QT, (stl + 1) * QT)
            ps = pool.tile([P, QT], F32, tag=tag)
            for kc in range(N_KCH):
                nc.tensor.matmul(
                    ps[:],
                    w_s[name][:, kc * DSL:(kc + 1) * DSL],
                    xtiles[kc][:, xsl],
                    start=(kc == 0),
                    stop=(kc == N_KCH - 1),
                )
            # bias add + bf16 cast on VectorE (keeps ScalarE free for exp)
            nc.vector.tensor_add(
                dst[:, sl], ps[:], b_s[name][:].to_broadcast((P, QT))
            )

        def project_v(st, xtiles, stl):
            """V projection s-tile, split per head into the padded VTh."""
            sl = slice(st * QT, (st + 1) * QT)
            xsl = slice(stl * QT, (stl + 1) * QT)
            ps = ps_acc.tile([P, QT], F32, tag="pv")
            for kc in range(N_KCH):
                nc.tensor.matmul(
                    ps[:],
                    w_s["v"][:, kc * DSL:(kc + 1) * DSL],
                    xtiles[kc][:, xsl],
                    start=(kc == 0),
                    stop=(kc == N_KCH - 1),
                )
            for h in range(HPC):
                hs = slice(h * DH, (h + 1) * DH)
                nc.vector.tensor_add(
                    VTh[h][0:DH, sl], ps[hs, :],
                    b_s["v"][hs, 0:1].to_broadcast((DH, QT)),
                )

        def proj_kv_quarter(qtr, kt, vt):
            for stl in range(QC // QT):
                st = qtr * (QC // QT) + stl
                project("k", KTs, st, kt, stl, pool=ps_acc, tag="pv")
                project_v(st, vt, stl)
                # one xbar transpose per (head, s-tile pair): out chunk j
                # gets k-positions at partition s%128; batched 1024-wide so
                # only 8 transposes sit on the scalar queue ahead of the exps
                if st % 2 == 1:
                    c0 = (st - 1) * (QT // KC)
                    for h in range(HPC):
                        nc.scalar.dma_start(
                            Vnat[h][:, c0:c0 + 2 * (QT // KC), 0:DH + 16],
                            VTh[h][:, (st - 1) * QT:(st + 1) * QT],
                            transpose=True,
                        )

        # ---- phase 1: K,V projections; V -> natural via DMA transpose ----
        qtiles = [None] * N_GRP
        for qtr in range(N_GRP):
            kt = load_quarter(xc, "xk", xkT, qtr, nsplit=3)
            vt = load_quarter(xc, "xv", xvT, qtr, nsplit=3)
            if qtr >= 1 and qtr - 1 < N_GRP:
                # prefetch Q quarters 0,1,2 while late K/V quarters stream
                qtiles[qtr - 1] = load_quarter(xq, "xq", xqT, qtr - 1)
            proj_kv_quarter(qtr, kt, vt)

        # ---- phase 2: Q proj + attention + output projection, per quarter --
        def project_q(qtr, qt):
            # borrows sc-pool slots; 128x128-mode matmuls (2 mode switches)
            for stl in range(QC // QT):
                project("q", QTs, qtr * (QC // QT) + stl, qt, stl,
                        pool=ps_sc, tag="sc")

        # PV split-K issue order: (tile, h, j); T0 = k-rows 0:64 on PE tile
        # (0,0), T1 = k-rows 64:128 on tile (64,0). Any two writes to the
        # same accumulator bank are >= 2 matmul slots apart, so the two
        # concurrently-streaming row tiles never touch one bank at once.
        PV_SEQ = [(0, 0, 0), (1, 1, 1), (0, 0, 1), (1, 1, 0),
                  (0, 1, 1), (1, 0, 0), (0, 1, 0), (1, 0, 1)]

        SPLITK_PV = False

        def pv_bundle(kc, accs, ets):
            if SPLITK_PV:
                for idx, (t, h, j) in enumerate(PV_SEQ):
                    rows = slice(t * DH, (t + 1) * DH)
                    nc.tensor.matmul(
                        accs[(h, j)][:],
                        Vnat[h][rows, kc, 0:DH + 1],
                        ets[h][rows, j * QT:(j + 1) * QT],
                        start=(kc == 0 and idx < 4),
                        stop=(kc == N_KC - 1 and idx >= 4),
                    )
            else:
                for h in range(HPC):
                    for j in range(QPH):
                        nc.tensor.matmul(
                            accs[(h, j)][:],
                            Vnat[h][:, kc, 0:DH + 1],
                            ets[h][:, j * QT:(j + 1) * QT],
                            start=(kc == 0), stop=(kc == N_KC - 1),
                        )

        def attn_finish(grp, accs):
            # normalize: attnT[hs, q] = acc[0:DH] * (1/acc[DH])
            q0 = grp * QC
            for h in range(HPC):
                hs = slice(h * DH, (h + 1) * DH)
                for j in range(QPH):
                    qsl = slice(q0 + j * QT, q0 + (j + 1) * QT)
                    pv = accs[(h, j)]
                    den = small.tile([1, QT], F32, tag="den")
                    nc.vector.tensor_copy(den[:], pv[DH:DH + 1, :])
                    rec = small.tile([1, QT], F32, tag="rec")
                    # approx_fast needs an SBUF source (PSUM input misreads)
                    nc.vector.reciprocal_approx_fast(rec[:], den[:])
                    bc = small.tile([DH, QT], F32, tag="bc")
                    nc.gpsimd.partition_broadcast(bc[:], rec[:])
                    nc.vector.tensor_mul(attnT[hs, qsl], pv[0:DH, :], bc[:])

        STORE_ENG = [lambda: nc.sync, lambda: nc.gpsimd]

        def outproj_batch(grp, b, tail=False):
            # 4 of the quarter's 16 (s-chunk, n-tile) output blocks; po tiles
            # borrow sc-pool slots. On the tail (last quarter) ScalarE is
            # exp-free, so split evictions between VectorE and ScalarE.
            for i in range(4):
                u = b * 4 + i
                st = grp * (QC // P) + u // 2
                nt = u % 2
                po = ps_sc.tile([P, QT], F32, tag="sc", name=f"po{grp}{u}")
                nc.tensor.matmul(
                    po[:],
                    attnT[:, st * P:(st + 1) * P],
                    wo_s[:, nt * QT:(nt + 1) * QT],
                    start=True, stop=True,
                )
                ot = oevict.tile([P, QT], BF16, tag="ot")
                if tail and i % 2 == 1:
                    nc.scalar.copy(ot[:], po[:])
                else:
                    nc.vector.tensor_copy(ot[:], po[:])
                eng = STORE_ENG[u % 2]() if not tail else (
                    nc.scalar if i % 2 == 0 else nc.sync
                )
                eng.dma_start(
                    out_d[st * P:(st + 1) * P, nt * QT:(nt + 1) * QT], ot[:]
                )

        def attn_group(grp, accs, pending):
            q0 = grp * QC
            et_prev = None
            for kc in range(N_KC):
                ks = slice(kc * KC, (kc + 1) * KC)
                ets = []
                for h in range(HPC):
                    hs = slice(h * DH, (h + 1) * DH)
                    sc = ps_sc.tile([P, QC], F32, tag="sc")
                    for j in range(QPH):
                        nc.tensor.matmul(
                            sc[:, j * QT:(j + 1) * QT],
                            KTs[hs, ks],
                            QTs[hs, q0 + j * QT:q0 + (j + 1) * QT],
                            start=True, stop=True,
                        )
                    et = et_pool.tile([P, QC], BF16, tag="et")
                    nc.scalar.activation(
                        et[:], sc[:],
                        mybir.ActivationFunctionType.Exp,
                        bias=mb_s[:, kc:kc + 1], scale=0.125,
                    )
                    ets.append(et)
                if et_prev is not None:
                    pv_bundle(kc - 1, accs, et_prev)
                et_prev = ets
                # sprinkle deferred PE work (prev quarter's output proj)
                if pending and kc % 4 == 3:
                    pending.pop(0)()
                if kc == 17 and grp + 1 < N_GRP:
                    project_q(grp + 1, qtiles[grp + 1])
            pv_bundle(N_KC - 1, accs, et_prev)

        project_q(0, qtiles[0])
        pending = []
        for grp in range(N_GRP):
            if grp + 1 < N_GRP and qtiles[grp + 1] is None:
                qtiles[grp + 1] = load_quarter(xq, "xq", xqT, grp + 1)
            accs = {
                (h, j): ps_acc.tile(
                    [DH + 1, QT], F32, tag="pv", name=f"acc{grp}{h}{j}"
                )
                for h in range(HPC) for j in range(QPH)
            }
            attn_group(grp, accs, pending)
            attn_finish(grp, accs)
            if grp < N_GRP - 1:
                pending = [
                    (lambda g=grp, b=b: outproj_batch(g, b)) for b in range(4)
                ]
            else:
                for fn in pending:  # leftovers (shouldn't happen)
                    fn()
                for b in range(4):
                    outproj_batch(grp, b, tail=True)

    nc.compile()
    _CACHE[key] = nc
    return nc


def _prep_in_maps(query, key, value, mask, Wq, bq, Wk, bk, Wv, bv, Wo, bo):
    f = np.float32
    qT = np.ascontiguousarray(np.asarray(query, dtype=f)[0].T).astype(BF)
    kT = np.ascontiguousarray(np.asarray(key, dtype=f)[0].T).astype(BF)
    vT = np.ascontiguousarray(np.asarray(value, dtype=f)[0].T).astype(BF)
    mb = np.where(np.asarray(mask)[0] == 0, f(-1e9), f(0.0)).astype(f)
    mb = np.ascontiguousarray(mb.reshape(N_KC, KC).T)  # [128, 32]
    WqT, WkT, WvT, WoT = (
        np.ascontiguousarray(np.asarray(W, dtype=f).T).astype(BF)
        for W in (Wq, Wk, Wv, Wo)
    )
    in_maps = []
    for c in range(NCORES):
        cs = slice(c * DSL, (c + 1) * DSL)
        in_maps.append({
            "xqT": qT, "xkT": kT, "xvT": vT,
            "wqT": np.ascontiguousarray(WqT[:, cs]).reshape(N_KCH, P, DSL),
            "wkT": np.ascontiguousarray(WkT[:, cs]).reshape(N_KCH, P, DSL),
            "wvT": np.ascontiguousarray(WvT[:, cs]).reshape(N_KCH, P, DSL),
            "woT": np.ascontiguousarray(WoT[cs, :]),
            "bq": np.ascontiguousarray(bq[cs].astype(f, copy=False)).reshape(DSL, 1),
            "bk": np.ascontiguousarray(bk[cs].astype(f, copy=False)).reshape(DSL, 1),
            "bv": np.ascontiguousarray(bv[cs].astype(f, copy=False)).reshape(DSL, 1),
            "mbias": mb,
        })
    return in_maps


def run(inputs, trace=False, trace_kwargs=None):
    nc = _build_nc()
    in_maps = _prep_in_maps(**inputs)
    res = run_bass_kernel_spmd(
        nc, in_maps, core_ids=list(range(NCORES)), trace=trace,
        **(trace_kwargs or {}),
    )
    bo = np.asarray(inputs["bo"], dtype=np.float32)
    acc = np.zeros((S, D), dtype=np.float32)
    for r in res.results:
        acc += np.asarray(r["out"], dtype=np.float32)
    out = (acc + bo[None, :]).astype(np.float32)[None]
    return out, res


def kernel(**inputs):
    out, _ = run(inputs, trace=False)
    return out


# revision 11
# speedup vs baseline: 1.0107x; 1.0107x over previous
"""GroupedQueryAttention (B=1, S=4096, D=1024, G=16 heads, DH=64) on 8 TRN2 NeuronCores.

Sharding: tensor-parallel over heads. Core c computes heads {2c, 2c+1}:
  - Q/K/V projections with column-sliced weights (128 out-dims per core),
    producing Q^T/K^T in [dout, seq] layout (host pre-transposes inputs).
  - V^T is built per head padded to 80 rows with an all-ones row 64; SBUF->
    SBUF xbar DMA transposes turn it into V-natural [kpos, 80] chunks whose
    col 64 is the ones column that makes the PV matmul accumulate the
    softmax denominator for free.
  - Flash-style attention without max-subtraction (scores are tiny:
    |s/8| < ~3), exp on ScalarE with fused 1/8 scale + per-key mask bias.
  - Output projection with row-sliced Wo produces a partial (4096, 1024)
    bf16 output per core; host sums the 8 partials and adds bo.

Attention engine schedule: the exp stream on ScalarE is the critical path
(2 x [128,1024] ACTIVATEs per 128-wide k-chunk, ~2.2us), so the PE work is
arranged to stay under that pace and never stall the ACT queue:
  - Scores for head0/head1 run as row-tiled (64x128) matmuls on PE tiles
    (0,0) and (64,0) (auto-derived from base partitions) and execute
    CONCURRENTLY on the two 64-row halves of the array.
  - PV runs split-K in the same (64,128) mode (no PE mode-switch drains in
    the attention loop): each accumulator takes a low-k-rows matmul from
    tile (0,0) and a high-k-rows matmul from tile (64,0). Issue order
    T0:[A00 A01 A11 A10] / T8:[A11 A10 A00 A01] keeps any two same-bank
    writes >= 2 matmul slots apart (row tiles must not touch one PSUM bank
    simultaneously).
  - PSUM: 2 score slots x 2 banks + 4 accumulators x 1 bank = 8 banks.
    The output projection / next-quarter Q projection borrow score-pool
    slots in small batches sprinkled between k-chunks of the following
    quarter, so ACT never waits at quarter boundaries.

All matmul operands are bf16 (fp32 PSUM accumulation). K/V inputs stream
through quarter-chunk tiles; Q streams in two quarters ahead.
"""

import os
import sys

for _p in ("/opt/trn_rl_repo", "/root/.axon_site/_ro/trn_rl_repo"):
    if os.path.isdir(_p) and _p not in sys.path:
        sys.path.insert(0, _p)

from contextlib import ExitStack

import ml_dtypes
import numpy as np

import concourse.bass as bass
import concourse.mybir as mybir
import concourse.tile as tile
from concourse import bacc
from concourse.bass_utils import run_bass_kernel_spmd

S = 4096          # sequence length
D = 1024          # model dim
G = 16            # heads
DH = 64           # head dim
P = 128           # partitions
QT = 512          # q-tile (moving free dim)
KC = 128          # k-chunk
NCORES = 8
HPC = G // NCORES             # heads per core = 2
N_ST = S // QT                # 8 s-tiles of 512
N_KCH = D // P                # 8 contraction chunks for projections
N_KC = S // KC                # 32 k-chunks for attention
DSL = P                       # per-core dout slice (2 heads * 64)
QPH = 2                       # q-tiles per attention group (quarter)
N_GRP = N_ST // QPH           # 4 quarters
QC = S // N_GRP               # 1024 columns per quarter chunk

F32 = mybir.dt.float32
BF16 = mybir.dt.bfloat16
BF = ml_dtypes.bfloat16

_CACHE = {}


def _build_nc():
    key = "nc"
    if key in _CACHE:
        return _CACHE[key]

    nc = bacc.Bacc(
        "TRN2", target_bir_lowering=False, debug=False, num_devices=NCORES
    )

    xqT = nc.dram_tensor("xqT", [D, S], BF16, kind="ExternalInput").ap()
    xkT = nc.dram_tensor("xkT", [D, S], BF16, kind="ExternalInput").ap()
    xvT = nc.dram_tensor("xvT", [D, S], BF16, kind="ExternalInput").ap()
    wqT = nc.dram_tensor("wqT", [N_KCH, P, DSL], BF16, kind="ExternalInput").ap()
    wkT = nc.dram_tensor("wkT", [N_KCH, P, DSL], BF16, kind="ExternalInput").ap()
    wvT = nc.dram_tensor("wvT", [N_KCH, P, DSL], BF16, kind="ExternalInput").ap()
    woT = nc.dram_tensor("woT", [DSL, D], BF16, kind="ExternalInput").ap()
    bq = nc.dram_tensor("bq", [DSL, 1], F32, kind="ExternalInput").ap()
    bk = nc.dram_tensor("bk", [DSL, 1], F32, kind="ExternalInput").ap()
    bv = nc.dram_tensor("bv", [DSL, 1], F32, kind="ExternalInput").ap()
    mbias = nc.dram_tensor("mbias", [P, N_KC], F32, kind="ExternalInput").ap()
    out_d = nc.dram_tensor("out", [S, D], BF16, kind="ExternalOutput").ap()

    with tile.TileContext(nc) as tc, ExitStack() as ctx:
        consts = ctx.enter_context(tc.tile_pool(name="consts", bufs=1))
        big = ctx.enter_context(tc.tile_pool(name="big", bufs=1))
        et_pool = ctx.enter_context(tc.tile_pool(name="et", bufs=8))
        small = ctx.enter_context(tc.tile_pool(name="small", bufs=2))
        oevict = ctx.enter_context(tc.tile_pool(name="oevict", bufs=4))
        # streamed K/V quarter-chunk tiles: 20 slots per tensor tag = 2.5
        # quarters of lookahead so chunk DMAs stay ahead of the proj matmuls
        xc = ctx.enter_context(tc.tile_pool(name="xc", bufs=20))
        # streamed Q quarter-chunk tiles, two quarters in flight
        xq = ctx.enter_context(tc.tile_pool(name="xq", bufs=16))
        # PSUM: 'sc' 2 slots x 2 banks (scores; also borrowed by out/q proj)
        #     + 'acc' 4 slots x 1 bank (PV accumulators; phase-1 proj tiles)
        ps_sc = ctx.enter_context(tc.tile_pool(name="ps_sc", bufs=2, space="PSUM"))
        ps_acc = ctx.enter_context(tc.tile_pool(name="ps_acc", bufs=4, space="PSUM"))

        # ---- constants (scalar-queue DMAs, parallel to x loads on sync) ----
        w_s = {}
        for name, wd in (("k", wkT), ("v", wvT), ("q", wqT)):
            w = consts.tile([P, N_KCH * DSL], BF16, tag=f"w{name}")
            for kc in range(N_KCH):
                nc.scalar.dma_start(w[:, kc * DSL:(kc + 1) * DSL], wd[kc])
            w_s[name] = w
        wo_s = consts.tile([DSL, D], BF16, tag="wo")
        nc.scalar.dma_start(wo_s[:], woT)
        b_s = {}
        for name, bd in (("q", bq), ("k", bk), ("v", bv)):
            b = consts.tile([DSL, 1], F32, tag=f"b{name}")
            nc.scalar.dma_start(b[:], bd)
            b_s[name] = b
        mb_s = consts.tile([P, N_KC], F32, tag="mb")
        nc.scalar.dma_start(mb_s[:], mbias)

        def load_quarter(pool, tag, xd, qtr, nsplit=2):
            """Load one quarter's 8 contraction chunks, rotating across
            DMA queues so the streams run in parallel. The scalar queue is
            only borrowed during phase 1 (before the exp stream starts)."""
            engines = [nc.sync, nc.gpsimd, nc.scalar][:nsplit]
            qsl = slice(qtr * QC, (qtr + 1) * QC)
            tiles = []
            for kc in range(N_KCH):
                t = pool.tile([P, QC], BF16, tag=tag, name=f"{tag}{qtr}{kc}")
                engines[kc % nsplit].dma_start(t[:], xd[kc * P:(kc + 1) * P, qsl])
                tiles.append(t)
            return tiles

        # ---- resident activations ----
        QTs = big.tile([P, S], BF16, tag="QTs")      # Q^T  [dout, s]
        KTs = big.tile([P, S], BF16, tag="KTs")      # K^T  [dout, s]
        # V^T per head, padded to 80 rows: rows 0-63 V data, row 64 ones
        # (becomes the PV ones-column after transpose -> softmax denom),
        # rows 65-79 zero pad (xbar transpose needs partition%16==0).
        VTh = [
            big.tile([DH + 16, S], BF16, tag=f"VTh{h}", name=f"VTh{h}")
            for h in range(HPC)
        ]
        # V natural per head: [k-part, chunk, 80]; col DH is the ones column.
        Vnat = [
            big.tile([P, N_KC, DH + 16], BF16, tag=f"Vn{h}", name=f"Vnat{h}")
            for h in range(HPC)
        ]
        attnT = big.tile([P, S], BF16, tag="attnT")  # normalized attn^T [din, s]

        for h in range(HPC):
            nc.vector.memset(VTh[h][DH:DH + 16, :], 0.0)
            nc.vector.memset(VTh[h][DH:DH + 1, :], 1.0)

        def project(name, dst, st, xtiles, stl, pool, tag):
            """One 512-wide s-tile of the `name` projection into dst (bf16)."""
            sl = slice(st * QT, (st + 1) * QT)
            xsl = slice(stl * QT, (stl + 1) * QT)
            ps = pool.tile([P, QT], F32, tag=tag)
            for kc in range(N_KCH):
                nc.tensor.matmul(
                    ps[:],
                    w_s[name][:, kc * DSL:(kc + 1) * DSL],
                    xtiles[kc][:, xsl],
                    start=(kc == 0),
                    stop=(kc == N_KCH - 1),
                )
            # bias add + bf16 cast on VectorE (keeps ScalarE free for exp)
            nc.vector.tensor_add(
                dst[:, sl], ps[:], b_s[name][:].to_broadcast((P, QT))
            )

        def project_v(st, xtiles, stl):
            """V projection s-tile, split per head into the padded VTh."""
            sl = slice(st * QT, (st + 1) * QT)
            xsl = slice(stl * QT, (stl + 1) * QT)
            ps = ps_acc.tile([P, QT], F32, tag="pv")
            for kc in range(N_KCH):
                nc.tensor.matmul(
                    ps[:],
                    w_s["v"][:, kc * DSL:(kc + 1) * DSL],
                    xtiles[kc][:, xsl],
                    start=(kc == 0),
                    stop=(kc == N_KCH - 1),
                )
            for h in range(HPC):
                hs = slice(h * DH, (h + 1) * DH)
                nc.vector.tensor_add(
                    VTh[h][0:DH, sl], ps[hs, :],
                    b_s["v"][hs, 0:1].to_broadcast((DH, QT)),
                )

        def proj_kv_quarter(qtr, kt, vt):
            for stl in range(QC // QT):
                st = qtr * (QC // QT) + stl
                project("k", KTs, st, kt, stl, pool=ps_acc, tag="pv")
                project_v(st, vt, stl)
                # one xbar transpose per (head, s-tile pair): out chunk j
                # gets k-positions at partition s%128; batched 1024-wide so
                # only 8 transposes sit on the scalar queue ahead of the exps
                if st % 2 == 1:
                    c0 = (st - 1) * (QT // KC)
                    for h in range(HPC):
                        nc.scalar.dma_start(
                            Vnat[h][:, c0:c0 + 2 * (QT // KC), 0:DH + 16],
                            VTh[h][:, (st - 1) * QT:(st + 1) * QT],
                            transpose=True,
                        )

        # ---- phase 1: K,V projections; V -> natural via DMA transpose ----
        qtiles = [None] * N_GRP
        for qtr in range(N_GRP):
            kt = load_quarter(xc, "xk", xkT, qtr, nsplit=3)
            vt = load_quarter(xc, "xv", xvT, qtr, nsplit=3)
            if qtr == 1:
                # prefetch Q quarter 0 while late K/V quarters stream
                qtiles[0] = load_quarter(xq, "xq", xqT, 0)
            proj_kv_quarter(qtr, kt, vt)

        # ---- phase 2: Q proj + attention + output projection, per quarter --
        def project_q(qtr, qt):
            # borrows sc-pool slots; 128x128-mode matmuls (2 mode switches)
            for stl in range(QC // QT):
                project("q", QTs, qtr * (QC // QT) + stl, qt, stl,
                        pool=ps_sc, tag="sc")

        # PV split-K issue order: (tile, h, j); T0 = k-rows 0:64 on PE tile
        # (0,0), T1 = k-rows 64:128 on tile (64,0). Any two writes to the
        # same accumulator bank are >= 2 matmul slots apart, so the two
        # concurrently-streaming row tiles never touch one bank at once.
        PV_SEQ = [(0, 0, 0), (1, 1, 1), (0, 0, 1), (1, 1, 0),
                  (0, 1, 1), (1, 0, 0), (0, 1, 0), (1, 0, 1)]

        SPLITK_PV = False

        def pv_bundle(kc, accs, ets):
            if SPLITK_PV:
                for idx, (t, h, j) in enumerate(PV_SEQ):
                    rows = slice(t * DH, (t + 1) * DH)
                    nc.tensor.matmul(
                        accs[(h, j)][:],
                        Vnat[h][rows, kc, 0:DH + 1],
                        ets[h][rows, j * QT:(j + 1) * QT],
                        start=(kc == 0 and idx < 4),
                        stop=(kc == N_KC - 1 and idx >= 4),
                    )
            else:
                for h in range(HPC):
                    for j in range(QPH):
                        nc.tensor.matmul(
                            accs[(h, j)][:],
                            Vnat[h][:, kc, 0:DH + 1],
                            ets[h][:, j * QT:(j + 1) * QT],
                            start=(kc == 0), stop=(kc == N_KC - 1),
                        )

        def attn_finish(grp, accs):
            # normalize: attnT[hs, q] = acc[0:DH] * (1/acc[DH])
            q0 = grp * QC
            for h in range(HPC):
                hs = slice(h * DH, (h + 1) * DH)
                for j in range(QPH):
                    qsl = slice(q0 + j * QT, q0 + (j + 1) * QT)
                    pv = accs[(h, j)]
                    den = small.tile([1, QT], F32, tag="den")
                    nc.vector.tensor_copy(den[:], pv[DH:DH + 1, :])
                    rec = small.tile([1, QT], F32, tag="rec")
                    # approx_fast needs an SBUF source (PSUM input misreads)
                    nc.vector.reciprocal_approx_fast(rec[:], den[:])
                    bc = small.tile([DH, QT], F32, tag="bc")
                    nc.gpsimd.partition_broadcast(bc[:], rec[:])
                    nc.vector.tensor_mul(attnT[hs, qsl], pv[0:DH, :], bc[:])

        STORE_ENG = [lambda: nc.sync, lambda: nc.gpsimd]

        def outproj_batch(grp, b, tail=False):
            # 4 of the quarter's 16 (s-chunk, n-tile) output blocks; po tiles
            # borrow sc-pool slots. On the tail (last quarter) ScalarE is
            # exp-free, so split evictions between VectorE and ScalarE.
            for i in range(4):
                u = b * 4 + i
                st = grp * (QC // P) + u // 2
                nt = u % 2
                po = ps_sc.tile([P, QT], F32, tag="sc", name=f"po{grp}{u}")
                nc.tensor.matmul(
                    po[:],
                    attnT[:, st * P:(st + 1) * P],
                    wo_s[:, nt * QT:(nt + 1) * QT],
                    start=True, stop=True,
                )
                ot = oevict.tile([P, QT], BF16, tag="ot")
                if tail and i % 2 == 1:
                    nc.scalar.copy(ot[:], po[:])
                else:
                    nc.vector.tensor_copy(ot[:], po[:])
                eng = STORE_ENG[u % 2]() if not tail else (
                    nc.scalar if i % 2 == 0 else nc.sync
                )
                eng.dma_start(
                    out_d[st * P:(st + 1) * P, nt * QT:(nt + 1) * QT], ot[:]
                )

        def attn_group(grp, accs, pending):
            q0 = grp * QC
            et_prev = None
            for kc in range(N_KC):
                ks = slice(kc * KC, (kc + 1) * KC)
                scs = [
                    ps_sc.tile([P, QC], F32, tag="sc", name=f"sc{h}")
                    for h in range(HPC)
                ]
                for j in range(QPH):
                    for h in range(HPC):
                        hs = slice(h * DH, (h + 1) * DH)
                        nc.tensor.matmul(
                            scs[h][:, j * QT:(j + 1) * QT],
                            KTs[hs, ks],
                            QTs[hs, q0 + j * QT:q0 + (j + 1) * QT],
                            start=True, stop=True,
                        )
                ets = []
                for h in range(HPC):
                    et = et_pool.tile([P, QC], BF16, tag="et")
                    nc.scalar.activation(
                        et[:], scs[h][:],
                        mybir.ActivationFunctionType.Exp,
                        bias=mb_s[:, kc:kc + 1], scale=0.125,
                    )
                    ets.append(et)
                if et_prev is not None:
                    pv_bundle(kc - 1, accs, et_prev)
                et_prev = ets
                # sprinkle deferred PE work (prev quarter's output proj)
                if pending and kc % 4 == 3:
                    pending.pop(0)()
                if kc == 17 and grp + 1 < N_GRP:
                    project_q(grp + 1, qtiles[grp + 1])
            pv_bundle(N_KC - 1, accs, et_prev)

        project_q(0, qtiles[0])
        pending = []
        for grp in range(N_GRP):
            if grp + 1 < N_GRP and qtiles[grp + 1] is None:
                qtiles[grp + 1] = load_quarter(xq, "xq", xqT, grp + 1)
            accs = {
                (h, j): ps_acc.tile(
                    [DH + 1, QT], F32, tag="pv", name=f"acc{grp}{h}{j}"
                )
                for h in range(HPC) for j in range(QPH)
            }
            attn_group(grp, accs, pending)
            attn_finish(grp, accs)
            if grp < N_GRP - 1:
                pending = [
                    (lambda g=grp, b=b: outproj_batch(g, b)) for b in range(4)
                ]
            else:
                for fn in pending:  # leftovers (shouldn't happen)
                    fn()
                for b in range(4):
                    outproj_batch(grp, b, tail=True)

    nc.compile()
    _CACHE[key] = nc
    return nc


def _prep_in_maps(query, key, value, mask, Wq, bq, Wk, bk, Wv, bv, Wo, bo):
    f = np.float32
    qT = np.ascontiguousarray(np.asarray(query, dtype=f)[0].T).astype(BF)
    kT = np.ascontiguousarray(np.asarray(key, dtype=f)[0].T).astype(BF)
    vT = np.ascontiguousarray(np.asarray(value, dtype=f)[0].T).astype(BF)
    mb = np.where(np.asarray(mask)[0] == 0, f(-1e9), f(0.0)).astype(f)
    mb = np.ascontiguousarray(mb.reshape(N_KC, KC).T)  # [128, 32]
    WqT, WkT, WvT, WoT = (
        np.ascontiguousarray(np.asarray(W, dtype=f).T).astype(BF)
        for W in (Wq, Wk, Wv, Wo)
    )
    in_maps = []
    for c in range(NCORES):
        cs = slice(c * DSL, (c + 1) * DSL)
        in_maps.append({
            "xqT": qT, "xkT": kT, "xvT": vT,
            "wqT": np.ascontiguousarray(WqT[:, cs]).reshape(N_KCH, P, DSL),
            "wkT": np.ascontiguousarray(WkT[:, cs]).reshape(N_KCH, P, DSL),
            "wvT": np.ascontiguousarray(WvT[:, cs]).reshape(N_KCH, P, DSL),
            "woT": np.ascontiguousarray(WoT[cs, :]),
            "bq": np.ascontiguousarray(bq[cs].astype(f, copy=False)).reshape(DSL, 1),
            "bk": np.ascontiguousarray(bk[cs].astype(f, copy=False)).reshape(DSL, 1),
            "bv": np.ascontiguousarray(bv[cs].astype(f, copy=False)).reshape(DSL, 1),
            "mbias": mb,
        })
    return in_maps


def run(inputs, trace=False, trace_kwargs=None):
    nc = _build_nc()
    in_maps = _prep_in_maps(**inputs)
    res = run_bass_kernel_spmd(
        nc, in_maps, core_ids=list(range(NCORES)), trace=trace,
        **(trace_kwargs or {}),
    )
    bo = np.asarray(inputs["bo"], dtype=np.float32)
    acc = np.zeros((S, D), dtype=np.float32)
    for r in res.results:
        acc += np.asarray(r["out"], dtype=np.float32)
    out = (acc + bo[None, :]).astype(np.float32)[None]
    return out, res


def kernel(**inputs):
    out, _ = run(inputs, trace=False)
    return out
